# revision 41
# baseline (speedup 1.0000x reference)
"""MetaSR (meta-upscale CNN) Trainium2 kernel, SPMD over 8 NeuronCores.

Algorithm (bilinear reformulation of the reference):
    feat = relu(conv5x5(x) + b)                      [N,64,H,W]
    hid  = relu(pos @ w1 + b1)                       [(H*s*W*s), 256]
    out[n,p,l,c] = sum_h hid[r(p,l),h] * U[n,l,h,c] + bias[n,l,c] + mean_c
      where U[n,l,h,c] = sum_k cols[n,l,k] * w2[h, k*3+c]   (k = 3x3 taps x 64 ch)
            bias[n,l,c] = sum_k cols[n,l,k] * b2[k*3+c]

Sharding: 8 horizontal strips of 16 image rows each (all of N on every core).

v5 design (176us/core vs 489us baseline):
  - host-side 6x6 im2col (+ones row for the folded conv bias): one [109,2340]
    fp16 DMA per image; conv = plain fp16 matmuls producing ft = [base;+1col]
    and fb = [base;+1row] directly (128 rows each) into one fp8 F tile.
    Halo zeroing via col memsets + per-partition row-mask multiplies.
  - MLP layer 1: posT [4,8192] fp16 loaded whole; plain fp16 matmuls.
  - stage B (U^T [128h, pix] psum tiles, fp8 DoubleRow): 3 matmuls per
    512-pix tile: k-tile pairs (taps01+34), (taps67+25), (tap8+zero) via
    strided 2nd-k-tile access patterns into the combined [ft|fb] tile.
  - per-pixel bias: compact M=4 DoubleRow matmuls -> [4,512] psum; ACT
    eviction adds the RGB means (per-partition bias) -> bs fp16; a K=4
    selector matmul broadcasts row cc of bs onto the output psum, then 8
    fp16 ones-matmuls (tile_position col packing) accumulate the DVE
    product pt = us * hidT reduced over h.
"""
import os
import numpy as np

SCALE = 2
RGB_MEAN = (0.4488, 0.4371, 0.404)
N, C, H, W = 4, 3, 128, 128
G0 = 64
NCORES = 8
HS = H // NCORES          # image rows per core (16)
FR = HS + 2               # feat rows incl unfold halo (18)
FC = W + 2                # feat cols incl unfold halo (130)
FREE = FR * FC            # 2340
XR = HS + 7               # padded x rows per core (23)
XW = W + 7                # padded x cols (135)
HH = 256                  # MLP hidden
LP = HS * W               # pixels per core (2048)
PR = 4 * LP               # pos rows per core (8192)
KC = 109                  # conv im2col K (3*36 taps + ones row)
WM = 800                  # w2p M cols: 6x128 U-blocks + 32 (4 bias + pad)

_CACHE = {}


def _build_nc():
    import concourse.bass as bass
    import concourse.tile as tile
    from concourse import bacc, mybir

    f32 = mybir.dt.float32
    f16 = mybir.dt.float16
    f8 = mybir.dt.float8e4
    DR = mybir.MatmulPerfMode.DoubleRow

    nc = bacc.Bacc("TRN2", target_bir_lowering=False, debug=False,
                   num_devices=NCORES)

    xim = nc.dram_tensor("xim", [N, KC, FREE], f8, kind="ExternalInput").ap()
    posT = nc.dram_tensor("posT", [4, PR], f16, kind="ExternalInput").ap()
    cwr = nc.dram_tensor("cwr", [2, KC, 128], f8, kind="ExternalInput").ap()
    w1a = nc.dram_tensor("w1a", [4, HH], f16, kind="ExternalInput").ap()
    w2p = nc.dram_tensor("w2p", [3, 128, 2 * WM], f8,
                         kind="ExternalInput").ap()
    rmask = nc.dram_tensor("rmask", [128, 8], f32, kind="ExternalInput").ap()
    sel3 = nc.dram_tensor("sel3", [4, 384], f16, kind="ExternalInput").ap()
    mean4 = nc.dram_tensor("mean4", [4, 1], f32, kind="ExternalInput").ap()
    ones16 = nc.dram_tensor("ones16", [128, 32], f16,
                            kind="ExternalInput").ap()
    out = nc.dram_tensor("out", [N, 3, 4, LP], f32, kind="ExternalOutput").ap()

    with tile.TileContext(nc) as tc:
        with tc.tile_pool(name="const", bufs=1) as cpool, \
             tc.tile_pool(name="feat", bufs=1) as fpool, \
             tc.tile_pool(name="hid", bufs=1) as hpool, \
             tc.tile_pool(name="im2col", bufs=2) as xpool, \
             tc.tile_pool(name="usb", bufs=3) as upool, \
             tc.tile_pool(name="pt", bufs=4) as ppool, \
             tc.tile_pool(name="bs", bufs=4) as bspool, \
             tc.tile_pool(name="ob", bufs=4) as obpool, \
             tc.tile_pool(name="bps", bufs=5, space="PSUM") as bps, \
             tc.tile_pool(name="ops", bufs=3, space="PSUM") as ops:

            # ---- constants to SBUF ----
            w1a_t = cpool.tile([4, HH], f16, tag="w1a")
            nc.scalar.dma_start(w1a_t[:], w1a[:])
            posT_t = cpool.tile([4, PR], f16, tag="posT")
            for (c0, c1), eng in (((0, 512), nc.scalar),
                                  ((512, 1024), nc.scalar),
                                  ((1024, 3072), nc.gpsimd),
                                  ((3072, 5120), nc.scalar),
                                  ((5120, 8192), nc.gpsimd)):
                eng.dma_start(posT_t[:, c0:c1], posT[:, c0:c1])
            cwr_t = []
            for v in range(2):
                t = cpool.tile([KC, 128], f8, tag=f"cwr{v}")
                nc.scalar.dma_start(t[:], cwr[v])
                cwr_t.append(t)
            w2p_t = []
            for p in range(3):
                t = cpool.tile([128, 2 * WM], f8, tag=f"w2p{p}")
                nc.gpsimd.dma_start(t[:], w2p[p])
                w2p_t.append(t)
            rmask_t = cpool.tile([128, 8], f32, tag="rmask")
            nc.gpsimd.dma_start(rmask_t[:], rmask[:])
            sel3_t = cpool.tile([4, 384], f16, tag="sel3")
            nc.gpsimd.dma_start(sel3_t[:], sel3[:])
            mean4_t = cpool.tile([4, 1], f32, tag="mean4")
            nc.gpsimd.dma_start(mean4_t[:], mean4[:])
            ones_t = cpool.tile([128, 32], f16, tag="ones16")
            nc.gpsimd.dma_start(ones_t[:], ones16[:])

            # ---- MLP layer 1 -> hidT fp16 tiles [128 h, 4096 (p,pix)] ----
            # posT column order (host): lp*4096 + p*1024 + (l % 1024)
            hidT = [None, None]
            for lp in range(2):
                hb = hpool.tile([128, 8192], f16, tag=f"hid_{lp}")
                for hch in range(2):
                    for cb in range(8):
                        ps = bps.tile([128, 512], f32, tag="B")
                        o = lp * 4096 + cb * 512
                        nc.tensor.matmul(
                            ps[:], w1a_t[:, hch * 128:(hch + 1) * 128],
                            posT_t[:, o:o + 512],
                            start=True, stop=True)
                        dst = hb[:, hch * 4096 + cb * 512:
                                 hch * 4096 + (cb + 1) * 512]
                        if (cb + hch) % 2 == 0:
                            nc.scalar.activation(
                                dst, ps[:],
                                mybir.ActivationFunctionType.Relu)
                        else:
                            nc.vector.tensor_scalar_max(dst, ps[:], 0.0)
                hidT[lp] = hb

            # ---- conv -> F = [ft | fb] fp8 ----
            dmae = [nc.sync, nc.vector, nc.scalar, nc.gpsimd]
            feat = []
            for n in range(N):
                F = fpool.tile([128, 2 * FREE], f8, tag=f"feat{n}")
                xt = xpool.tile([KC, FREE], f8, tag="x")
                for (p0, p1) in ((0, 32), (32, 64), (64, 96), (96, KC)):
                    nc.sync.dma_start(xt[p0:p1, :], xim[n][p0:p1, :])

                for v in range(2):  # 0: ft (base|+1col), 1: fb (base|+1row)
                    for ch in range(5):
                        o = ch * 512
                        nw = min(512, FREE - o)
                        ps = bps.tile([128, 512], f32, tag="B")
                        nc.tensor.matmul(ps[:, :nw], cwr_t[v][:],
                                         xt[:, o:o + nw],
                                         start=True, stop=True)
                        dst = F[:, v * FREE + o: v * FREE + o + nw]
                        if (ch + v) % 2 == 0:
                            nc.scalar.activation(
                                dst, ps[:, :nw],
                                mybir.ActivationFunctionType.Relu)
                        else:
                            nc.vector.tensor_scalar_max(dst, ps[:, :nw], 0.0)

                # halo zeroing: columns via memsets
                fa = F[:]
                flo = F[0:64, :]
                fhi = F[64:128, :]
                # ft lower: cols {0,129}; ft upper: cols {128,129}
                nc.gpsimd.memset(bass.AP(flo.tensor, flo.offset,
                                         [flo.ap[0], [FC, FR], [FC - 1, 2]]), 0)
                nc.gpsimd.memset(bass.AP(fhi.tensor, fhi.offset + FC - 2,
                                         [fhi.ap[0], [FC, FR], [1, 2]]), 0)
                # fb (both halves): cols {0,129}
                nc.gpsimd.memset(bass.AP(fa.tensor, fa.offset + FREE,
                                         [fa.ap[0], [FC, FR], [FC - 1, 2]]), 0)
                # halo rows via per-core per-partition masks
                fv = fa.rearrange("p (x q) -> p x q", q=FC)
                for (row, mcol) in ((0, 0), (FR - 1, 1),
                                    (FR, 2), (2 * FR - 1, 3), (2 * FR - 2, 4)):
                    nc.vector.tensor_scalar_mul(
                        fv[:, row:row + 1, :], fv[:, row:row + 1, :],
                        rmask_t[:, mcol:mcol + 1])
                feat.append(F)

            # stage-B / bias window pair APs into F = [ft | fb]
            def wpair(n, pr, r0):
                fa = feat[n][:]
                if pr == 0:
                    off, dlt = r0 * FC, FC
                elif pr == 1:
                    off, dlt = (r0 + 2) * FC, FREE + 2 - 2 * FC
                else:
                    # i=1 k-tile has zero weights; +FC keeps the read
                    # in-bounds (spills into the fb region for r0=12)
                    off, dlt = (r0 + 2) * FC + 2, FC
                return bass.AP(fa.tensor, fa.offset + off,
                               [fa.ap[0], [dlt, 2], [FC, 4], [1, 128]])

            w2v = [w2p_t[p][:].rearrange("p (i m) -> p i m", i=2)
                   for p in range(3)]

            # ---- main loop ----
            for n in range(N):
                for lp in range(2):
                    # per-pixel bias -> bs fp16 [4, 512] x2 (mean folded in)
                    bss = []
                    for half in range(2):
                        r0 = lp * 8 + half * 4
                        pb = bps.tile([128, 512], f32, tag="B")
                        for pr in range(3):
                            nc.tensor.matmul(
                                pb[0:32, :], w2v[pr][:, :, 768:800],
                                wpair(n, pr, r0),
                                start=(pr == 0), stop=(pr == 2),
                                perf_mode=DR)
                        bs = bspool.tile([4, 512], f16, tag="bs")
                        # bias+mean is always positive (mean >= 103, |bias|
                        # < ~3) so Relu is an identity; Copy rejects AP bias
                        nc.scalar.activation(
                            bs[:], pb[0:4, :],
                            mybir.ActivationFunctionType.Relu,
                            bias=mean4_t[:])
                        bss.append(bs)

                    pts = {}
                    for cc in range(3):
                        us = upool.tile([128, 2048], f16, tag="us")
                        for hch in range(2):
                            mb = cc * 2 + hch
                            for half in range(2):
                                r0 = lp * 8 + half * 4
                                pbk = bps.tile([128, 512], f32, tag="B")
                                for pr in range(3):
                                    nc.tensor.matmul(
                                        pbk[:],
                                        w2v[pr][:, :, mb * 128:(mb + 1) * 128],
                                        wpair(n, pr, r0),
                                        start=(pr == 0), stop=(pr == 2),
                                        perf_mode=DR)
                                nc.scalar.activation(
                                    us[:, hch * 1024 + half * 512:
                                       hch * 1024 + (half + 1) * 512],
                                    pbk[:],
                                    mybir.ActivationFunctionType.Copy)
                        pt = ppool.tile([128, 8192], f16, tag="pt")
                        nc.vector.tensor_mul(
                            pt[:].rearrange("p (i a q) -> p i a q",
                                            i=2, q=1024),
                            us[:].rearrange("p (i q) -> p i q", i=2)
                                .unsqueeze(2).broadcast_to((128, 2, 4, 1024)),
                            hidT[lp][:].rearrange("p (i a q) -> p i a q",
                                                  i=2, q=1024))
                        pts[cc] = pt

                    for cc in range(3):
                        for half in range(2):
                            po = ops.tile([128, 512], f32, tag="po")
                            nc.tensor.matmul(
                                po[:], sel3_t[:, cc * 128:(cc + 1) * 128],
                                bss[half][:],
                                start=True, stop=False,
                                skip_group_check=True)
                            for hch in range(2):
                                for p in range(4):
                                    sl = slice(hch * 4096 + p * 1024
                                               + half * 512,
                                               hch * 4096 + p * 1024
                                               + half * 512 + 512)
                                    nc.tensor.matmul(
                                        po[32 * p:32 * p + 32, :],
                                        ones_t[:, 0:32],
                                        pts[cc][:, sl],
                                        start=False,
                                        stop=(hch == 1 and p == 3),
                                        skip_group_check=True,
                                        tile_position=(0, 32 * p))
                            posb = obpool.tile([128, 512], f32, tag="posb")
                            if cc % 2 == 0:
                                nc.scalar.activation(
                                    posb[:], po[:],
                                    mybir.ActivationFunctionType.Copy)
                            else:
                                nc.vector.tensor_scalar_add(
                                    posb[:], po[:], 0.0)
                            posrc = posb[:].rearrange(
                                "(a b) q -> a b q", b=32)[:, 0, :]
                            oeng = nc.sync if (cc + half) % 2 == 0 \
                                else nc.gpsimd
                            oeng.dma_start(
                                out[n, cc][:, lp * 1024 + half * 512:
                                           lp * 1024 + half * 512 + 512],
                                posrc)

    nc.compile()
    return nc


def _host_prep(x, pos_mat, conv_w, conv_b, w1, b1, w2, b2):
    from concourse import mybir
    f = np.float32
    f8 = mybir.dt.np(mybir.dt.float8e4)

    # x: pad 3 top/left, 4 bottom/right for the 6x6 tap window
    xpad = np.pad(x, ((0, 0), (0, 0), (3, 4), (3, 4))).astype(f)

    # conv weights: [v, k(109), 128]; cols = [base(64) | shifted(64)];
    # row 108 = conv bias (pairs with the im2col ones row)
    def cw6(dr, dc):
        w6 = np.zeros((G0, C, 6, 6), f)
        w6[:, :, dr:dr + 5, dc:dc + 5] = conv_w
        return w6.transpose(1, 2, 3, 0).reshape(108, G0)

    base = cw6(0, 0)
    cwr = np.zeros((2, KC, 128), f)
    for v, shifted in enumerate((cw6(0, 1), cw6(1, 0))):
        cwr[v, :108, 0:G0] = base
        cwr[v, :108, G0:128] = shifted
        cwr[v, 108, 0:G0] = conv_b
        cwr[v, 108, G0:128] = conv_b

    w1a = np.vstack([w1, b1[None, :]]).astype(f)

    # w2 pair blocks [3, 128, 2(i), 772]
    Wr = w2.reshape(HH, 576, 3)
    b2r = b2.reshape(576, 3)

    def block_mat(tlo, thi):
        M = np.zeros((128, WM), f)
        for hf, t in ((0, tlo), (1, thi)):
            if t is None:
                continue
            kidx = np.arange(G0) * 9 + t
            for mb in range(6):
                cc, hch = mb // 2, mb % 2
                M[hf * 64:(hf + 1) * 64, mb * 128:(mb + 1) * 128] = \
                    Wr[hch * 128:(hch + 1) * 128, kidx, cc].T
            M[hf * 64:(hf + 1) * 64, 768:771] = b2r[kidx, :]
        return M

    BLOCKS = {0: (0, 1), 1: (3, 4), 2: (6, 7), 3: (2, 5), 4: (8, None)}
    PAIRS = [(0, 1), (2, 3), (4, None)]
    w2p = np.zeros((3, 128, 2, WM), f)
    for P, (b0, b1_) in enumerate(PAIRS):
        w2p[P, :, 0, :] = block_mat(*BLOCKS[b0])
        if b1_ is not None:
            w2p[P, :, 1, :] = block_mat(*BLOCKS[b1_])

    sel3 = np.zeros((4, 384), np.float16)
    for cc in range(3):
        sel3[cc, cc * 128:(cc + 1) * 128] = 1.0
    mean4 = np.zeros((4, 1), f)
    mean4[:3, 0] = np.asarray(RGB_MEAN, f) * 255.0
    ones16 = np.ones((128, 32), np.float16)

    in_maps = []
    for core in range(NCORES):
        xsl = xpad[:, :, HS * core: HS * core + XR, :]
        sw = np.lib.stride_tricks.sliding_window_view(
            xsl, (FR, FC), axis=(2, 3))          # [N, C, 6, 6, FR, FC]
        xim = np.empty((N, KC, FREE), f8)
        xim[:, :108, :] = sw.transpose(0, 1, 2, 3, 4, 5).reshape(
            N, C, 36, FREE).reshape(N, 108, FREE)
        xim[:, 108, :] = 1.0

        pos = pos_mat[0, PR * core: PR * (core + 1), :]
        pos = pos.reshape(2, 8, 2, W, 2, 3).transpose(0, 2, 4, 1, 3, 5) \
            .reshape(PR, 3)
        posTc = np.ascontiguousarray(
            np.concatenate([pos, np.ones((PR, 1), f)], 1).T)

        rm = np.ones((128, 8), f)
        if core == 0:
            rm[:, 0] = 0.0            # ft row 0
            rm[0:64, 2] = 0.0         # fb row 0, base half
        if core == NCORES - 1:
            rm[:, 1] = 0.0            # ft row 17
            rm[:, 3] = 0.0            # fb row 17
            rm[64:128, 4] = 0.0       # fb row 16, +1row half

        in_maps.append({"xim": xim, "posT": posTc.astype(np.float16),
                        "cwr": cwr.astype(f8),
                        "w1a": w1a.astype(np.float16),
                        "w2p": w2p.reshape(3, 128, 2 * WM).astype(f8),
                        "rmask": rm, "sel3": sel3, "mean4": mean4,
                        "ones16": ones16})
    return in_maps


def _assemble(results):
    full = np.empty((N, 3, H * SCALE, W * SCALE), np.float32)
    for core in range(NCORES):
        r = results[core]["out"].reshape(N, 3, 2, 2, HS, W)
        blk = r.transpose(0, 1, 4, 2, 5, 3).reshape(N, 3, HS * 2, W * 2)
        full[:, :, HS * 2 * core: HS * 2 * (core + 1), :] = blk
    return full


def kernel(**inputs):
    from concourse.bass_utils import run_bass_kernel_spmd
    if "nc" not in _CACHE:
        _CACHE["nc"] = _build_nc()
    in_maps = _host_prep(**inputs)
    res = run_bass_kernel_spmd(_CACHE["nc"], in_maps, list(range(NCORES)))
    _CACHE["last_result"] = res
    return _assemble(res.results)


# revision 43
# speedup vs baseline: 1.0024x; 1.0024x over previous
"""MetaSR (meta-upscale CNN) Trainium2 kernel, SPMD over 8 NeuronCores.

Algorithm (bilinear reformulation of the reference):
    feat = relu(conv5x5(x) + b)                      [N,64,H,W]
    hid  = relu(pos @ w1 + b1)                       [(H*s*W*s), 256]
    out[n,p,l,c] = sum_h hid[r(p,l),h] * U[n,l,h,c] + bias[n,l,c] + mean_c
      where U[n,l,h,c] = sum_k cols[n,l,k] * w2[h, k*3+c]   (k = 3x3 taps x 64 ch)
            bias[n,l,c] = sum_k cols[n,l,k] * b2[k*3+c]

Sharding: 8 horizontal strips of 16 image rows each (all of N on every core).

v5 design (176us/core vs 489us baseline):
  - host-side 6x6 im2col (+ones row for the folded conv bias): one [109,2340]
    fp16 DMA per image; conv = plain fp16 matmuls producing ft = [base;+1col]
    and fb = [base;+1row] directly (128 rows each) into one fp8 F tile.
    Halo zeroing via col memsets + per-partition row-mask multiplies.
  - MLP layer 1: posT [4,8192] fp16 loaded whole; plain fp16 matmuls.
  - stage B (U^T [128h, pix] psum tiles, fp8 DoubleRow): 3 matmuls per
    512-pix tile: k-tile pairs (taps01+34), (taps67+25), (tap8+zero) via
    strided 2nd-k-tile access patterns into the combined [ft|fb] tile.
  - per-pixel bias: compact M=4 DoubleRow matmuls -> [4,512] psum; ACT
    eviction adds the RGB means (per-partition bias) -> bs fp16; a K=4
    selector matmul broadcasts row cc of bs onto the output psum, then 8
    fp16 ones-matmuls (tile_position col packing) accumulate the DVE
    product pt = us * hidT reduced over h.
"""
import os
import numpy as np

SCALE = 2
RGB_MEAN = (0.4488, 0.4371, 0.404)
N, C, H, W = 4, 3, 128, 128
G0 = 64
NCORES = 8
HS = H // NCORES          # image rows per core (16)
FR = HS + 2               # feat rows incl unfold halo (18)
FC = W + 2                # feat cols incl unfold halo (130)
FREE = FR * FC            # 2340
XR = HS + 7               # padded x rows per core (23)
XW = W + 7                # padded x cols (135)
HH = 256                  # MLP hidden
LP = HS * W               # pixels per core (2048)
PR = 4 * LP               # pos rows per core (8192)
KC = 109                  # conv im2col K (3*36 taps + ones row)
WM = 800                  # w2p M cols: 6x128 U-blocks + 32 (4 bias + pad)

_CACHE = {}


def _build_nc():
    import concourse.bass as bass
    import concourse.tile as tile
    from concourse import bacc, mybir

    f32 = mybir.dt.float32
    f16 = mybir.dt.float16
    f8 = mybir.dt.float8e4
    DR = mybir.MatmulPerfMode.DoubleRow

    nc = bacc.Bacc("TRN2", target_bir_lowering=False, debug=False,
                   num_devices=NCORES)

    xim = nc.dram_tensor("xim", [N, KC, FREE], f8, kind="ExternalInput").ap()
    posT = nc.dram_tensor("posT", [4, PR], f16, kind="ExternalInput").ap()
    cwr = nc.dram_tensor("cwr", [2, KC, 128], f8, kind="ExternalInput").ap()
    w1a = nc.dram_tensor("w1a", [4, HH], f16, kind="ExternalInput").ap()
    w2p = nc.dram_tensor("w2p", [3, 128, 2 * WM], f8,
                         kind="ExternalInput").ap()
    rmask = nc.dram_tensor("rmask", [128, 8], f32, kind="ExternalInput").ap()
    sel3 = nc.dram_tensor("sel3", [4, 384], f16, kind="ExternalInput").ap()
    mean4 = nc.dram_tensor("mean4", [4, 1], f32, kind="ExternalInput").ap()
    ones16 = nc.dram_tensor("ones16", [128, 32], f16,
                            kind="ExternalInput").ap()
    out = nc.dram_tensor("out", [N, 3, 4, LP], f32, kind="ExternalOutput").ap()

    with tile.TileContext(nc) as tc:
        with tc.tile_pool(name="const", bufs=1) as cpool, \
             tc.tile_pool(name="feat", bufs=1) as fpool, \
             tc.tile_pool(name="hid", bufs=1) as hpool, \
             tc.tile_pool(name="im2col", bufs=2) as xpool, \
             tc.tile_pool(name="usb", bufs=5) as upool, \
             tc.tile_pool(name="pt", bufs=8) as ppool, \
             tc.tile_pool(name="bs", bufs=4) as bspool, \
             tc.tile_pool(name="ob", bufs=4) as obpool, \
             tc.tile_pool(name="bps", bufs=5, space="PSUM") as bps, \
             tc.tile_pool(name="ops", bufs=3, space="PSUM") as ops:

            # ---- PE pstate warmup: dummy matmuls with no DMA deps so the
            # array is at full clock when the real work's data arrives ----
            warm = cpool.tile([128, 512], f16, tag="warm")
            nc.vector.memset(warm[:], 1.0)
            for _ in range(24):
                wps = bps.tile([128, 512], f32, tag="B")
                nc.tensor.matmul(wps[:], warm[:, 0:128], warm[:],
                                 start=True, stop=True)

            # ---- constants to SBUF ----
            w1a_t = cpool.tile([4, HH], f16, tag="w1a")
            nc.scalar.dma_start(w1a_t[:], w1a[:])
            posT_t = cpool.tile([4, PR], f16, tag="posT")
            for ci, eng in enumerate((nc.scalar, nc.gpsimd, nc.scalar,
                                      nc.gpsimd)):
                eng.dma_start(posT_t[:, ci * 2048:(ci + 1) * 2048],
                              posT[:, ci * 2048:(ci + 1) * 2048])
            cwr_t = []
            for v in range(2):
                t = cpool.tile([KC, 128], f8, tag=f"cwr{v}")
                nc.scalar.dma_start(t[:], cwr[v])
                cwr_t.append(t)
            w2p_t = []
            for p in range(3):
                t = cpool.tile([128, 2 * WM], f8, tag=f"w2p{p}")
                nc.gpsimd.dma_start(t[:], w2p[p])
                w2p_t.append(t)
            rmask_t = cpool.tile([128, 8], f32, tag="rmask")
            nc.gpsimd.dma_start(rmask_t[:], rmask[:])
            sel3_t = cpool.tile([4, 384], f16, tag="sel3")
            nc.gpsimd.dma_start(sel3_t[:], sel3[:])
            mean4_t = cpool.tile([4, 1], f32, tag="mean4")
            nc.gpsimd.dma_start(mean4_t[:], mean4[:])
            ones_t = cpool.tile([128, 32], f16, tag="ones16")
            nc.gpsimd.dma_start(ones_t[:], ones16[:])

            # ---- MLP layer 1 -> hidT fp16 tiles [128 h, 4096 (p,pix)] ----
            # posT column order (host): lp*4096 + p*1024 + (l % 1024)
            hidT = [[None] * 2, [None] * 2]
            for lp in range(2):
                for hch in range(2):
                    hb = hpool.tile([128, 4096], f16, tag=f"hid{hch}_{lp}")
                    for cb in range(8):
                        ps = bps.tile([128, 512], f32, tag="B")
                        o = lp * 4096 + cb * 512
                        nc.tensor.matmul(
                            ps[:], w1a_t[:, hch * 128:(hch + 1) * 128],
                            posT_t[:, o:o + 512],
                            start=True, stop=True)
                        dst = hb[:, cb * 512:(cb + 1) * 512]
                        if (cb + hch) % 2 == 0:
                            nc.scalar.activation(
                                dst, ps[:],
                                mybir.ActivationFunctionType.Relu)
                        else:
                            nc.vector.tensor_scalar_max(dst, ps[:], 0.0)
                    hidT[hch][lp] = hb

            # ---- conv -> F = [ft | fb] fp8 ----
            dmae = [nc.sync, nc.vector, nc.scalar, nc.gpsimd]
            feat = []
            for n in range(N):
                F = fpool.tile([128, 2 * FREE], f8, tag=f"feat{n}")
                xt = xpool.tile([KC, FREE], f8, tag="x")
                for (p0, p1) in ((0, 32), (32, 64), (64, 96), (96, KC)):
                    nc.sync.dma_start(xt[p0:p1, :], xim[n][p0:p1, :])

                for v in range(2):  # 0: ft (base|+1col), 1: fb (base|+1row)
                    for ch in range(5):
                        o = ch * 512
                        nw = min(512, FREE - o)
                        ps = bps.tile([128, 512], f32, tag="B")
                        nc.tensor.matmul(ps[:, :nw], cwr_t[v][:],
                                         xt[:, o:o + nw],
                                         start=True, stop=True)
                        dst = F[:, v * FREE + o: v * FREE + o + nw]
                        if (ch + v) % 2 == 0:
                            nc.scalar.activation(
                                dst, ps[:, :nw],
                                mybir.ActivationFunctionType.Relu)
                        else:
                            nc.vector.tensor_scalar_max(dst, ps[:, :nw], 0.0)

                # halo zeroing: columns via memsets
                fa = F[:]
                flo = F[0:64, :]
                fhi = F[64:128, :]
                # ft lower: cols {0,129}; ft upper: cols {128,129}
                nc.gpsimd.memset(bass.AP(flo.tensor, flo.offset,
                                         [flo.ap[0], [FC, FR], [FC - 1, 2]]), 0)
                nc.gpsimd.memset(bass.AP(fhi.tensor, fhi.offset + FC - 2,
                                         [fhi.ap[0], [FC, FR], [1, 2]]), 0)
                # fb (both halves): cols {0,129}
                nc.gpsimd.memset(bass.AP(fa.tensor, fa.offset + FREE,
                                         [fa.ap[0], [FC, FR], [FC - 1, 2]]), 0)
                # halo rows via per-core per-partition masks
                fv = fa.rearrange("p (x q) -> p x q", q=FC)
                for (row, mcol) in ((0, 0), (FR - 1, 1),
                                    (FR, 2), (2 * FR - 1, 3), (2 * FR - 2, 4)):
                    nc.vector.tensor_scalar_mul(
                        fv[:, row:row + 1, :], fv[:, row:row + 1, :],
                        rmask_t[:, mcol:mcol + 1])
                feat.append(F)

            # stage-B / bias window pair APs into F = [ft | fb]
            def wpair(n, pr, r0):
                fa = feat[n][:]
                if pr == 0:
                    off, dlt = r0 * FC, FC
                elif pr == 1:
                    off, dlt = (r0 + 2) * FC, FREE + 2 - 2 * FC
                else:
                    # i=1 k-tile has zero weights; +FC keeps the read
                    # in-bounds (spills into the fb region for r0=12)
                    off, dlt = (r0 + 2) * FC + 2, FC
                return bass.AP(fa.tensor, fa.offset + off,
                               [fa.ap[0], [dlt, 2], [FC, 4], [1, 128]])

            w2v = [w2p_t[p][:].rearrange("p (i m) -> p i m", i=2)
                   for p in range(3)]

            # ---- main loop ----
            for n in range(N):
                for lp in range(2):
                    # per-pixel bias -> bs fp16 [4, 512] x2 (mean folded in)
                    bss = []
                    for half in range(2):
                        r0 = lp * 8 + half * 4
                        pb = bps.tile([128, 512], f32, tag="B")
                        for pr in range(3):
                            nc.tensor.matmul(
                                pb[0:32, :], w2v[pr][:, :, 768:800],
                                wpair(n, pr, r0),
                                start=(pr == 0), stop=(pr == 2),
                                perf_mode=DR)
                        bs = bspool.tile([4, 512], f16, tag="bs")
                        # bias+mean is always positive (mean >= 103, |bias|
                        # < ~3) so Relu is an identity; Copy rejects AP bias
                        nc.scalar.activation(
                            bs[:], pb[0:4, :],
                            mybir.ActivationFunctionType.Relu,
                            bias=mean4_t[:])
                        bss.append(bs)

                    pts = {}
                    for cc in range(3):
                        for hch in range(2):
                            mb = cc * 2 + hch
                            us = upool.tile([128, 1024], f16, tag="us")
                            for half in range(2):
                                r0 = lp * 8 + half * 4
                                pbk = bps.tile([128, 512], f32, tag="B")
                                for pr in range(3):
                                    nc.tensor.matmul(
                                        pbk[:],
                                        w2v[pr][:, :, mb * 128:(mb + 1) * 128],
                                        wpair(n, pr, r0),
                                        start=(pr == 0), stop=(pr == 2),
                                        perf_mode=DR)
                                nc.scalar.activation(
                                    us[:, half * 512:(half + 1) * 512],
                                    pbk[:],
                                    mybir.ActivationFunctionType.Copy)
                            pt = ppool.tile([128, 4096], f16, tag="pt")
                            nc.vector.tensor_mul(
                                pt[:].rearrange("p (a q) -> p a q", q=1024),
                                us[:].unsqueeze(1).broadcast_to((128, 4, 1024)),
                                hidT[hch][lp][:].rearrange(
                                    "p (a q) -> p a q", q=1024))
                            pts[(cc, hch)] = pt

                    for cc in range(3):
                        for half in range(2):
                            po = ops.tile([128, 512], f32, tag="po")
                            nc.tensor.matmul(
                                po[:], sel3_t[:, cc * 128:(cc + 1) * 128],
                                bss[half][:],
                                start=True, stop=False,
                                skip_group_check=True)
                            for hch in range(2):
                                for p in range(4):
                                    sl = slice(p * 1024 + half * 512,
                                               p * 1024 + half * 512 + 512)
                                    nc.tensor.matmul(
                                        po[32 * p:32 * p + 32, :],
                                        ones_t[:, 0:32],
                                        pts[(cc, hch)][:, sl],
                                        start=False,
                                        stop=(hch == 1 and p == 3),
                                        skip_group_check=True,
                                        tile_position=(0, 32 * p))
                            posb = obpool.tile([128, 512], f32, tag="posb")
                            if cc % 2 == 0:
                                nc.scalar.activation(
                                    posb[:], po[:],
                                    mybir.ActivationFunctionType.Copy)
                            else:
                                nc.vector.tensor_scalar_add(
                                    posb[:], po[:], 0.0)
                            posrc = posb[:].rearrange(
                                "(a b) q -> a b q", b=32)[:, 0, :]
                            oeng = nc.sync if (cc + half) % 2 == 0 \
                                else nc.gpsimd
                            oeng.dma_start(
                                out[n, cc][:, lp * 1024 + half * 512:
                                           lp * 1024 + half * 512 + 512],
                                posrc)

    nc.compile()
    return nc


def _host_prep(x, pos_mat, conv_w, conv_b, w1, b1, w2, b2):
    from concourse import mybir
    f = np.float32
    f8 = mybir.dt.np(mybir.dt.float8e4)

    # x: pad 3 top/left, 4 bottom/right for the 6x6 tap window
    xpad = np.pad(x, ((0, 0), (0, 0), (3, 4), (3, 4))).astype(f)

    # conv weights: [v, k(109), 128]; cols = [base(64) | shifted(64)];
    # row 108 = conv bias (pairs with the im2col ones row)
    def cw6(dr, dc):
        w6 = np.zeros((G0, C, 6, 6), f)
        w6[:, :, dr:dr + 5, dc:dc + 5] = conv_w
        return w6.transpose(1, 2, 3, 0).reshape(108, G0)

    base = cw6(0, 0)
    cwr = np.zeros((2, KC, 128), f)
    for v, shifted in enumerate((cw6(0, 1), cw6(1, 0))):
        cwr[v, :108, 0:G0] = base
        cwr[v, :108, G0:128] = shifted
        cwr[v, 108, 0:G0] = conv_b
        cwr[v, 108, G0:128] = conv_b

    w1a = np.vstack([w1, b1[None, :]]).astype(f)

    # w2 pair blocks [3, 128, 2(i), 772]
    Wr = w2.reshape(HH, 576, 3)
    b2r = b2.reshape(576, 3)

    def block_mat(tlo, thi):
        M = np.zeros((128, WM), f)
        for hf, t in ((0, tlo), (1, thi)):
            if t is None:
                continue
            kidx = np.arange(G0) * 9 + t
            for mb in range(6):
                cc, hch = mb // 2, mb % 2
                M[hf * 64:(hf + 1) * 64, mb * 128:(mb + 1) * 128] = \
                    Wr[hch * 128:(hch + 1) * 128, kidx, cc].T
            M[hf * 64:(hf + 1) * 64, 768:771] = b2r[kidx, :]
        return M

    BLOCKS = {0: (0, 1), 1: (3, 4), 2: (6, 7), 3: (2, 5), 4: (8, None)}
    PAIRS = [(0, 1), (2, 3), (4, None)]
    w2p = np.zeros((3, 128, 2, WM), f)
    for P, (b0, b1_) in enumerate(PAIRS):
        w2p[P, :, 0, :] = block_mat(*BLOCKS[b0])
        if b1_ is not None:
            w2p[P, :, 1, :] = block_mat(*BLOCKS[b1_])

    sel3 = np.zeros((4, 384), np.float16)
    for cc in range(3):
        sel3[cc, cc * 128:(cc + 1) * 128] = 1.0
    mean4 = np.zeros((4, 1), f)
    mean4[:3, 0] = np.asarray(RGB_MEAN, f) * 255.0
    ones16 = np.ones((128, 32), np.float16)

    in_maps = []
    for core in range(NCORES):
        xsl = xpad[:, :, HS * core: HS * core + XR, :]
        sw = np.lib.stride_tricks.sliding_window_view(
            xsl, (FR, FC), axis=(2, 3))          # [N, C, 6, 6, FR, FC]
        xim = np.empty((N, KC, FREE), f8)
        xim[:, :108, :] = sw.transpose(0, 1, 2, 3, 4, 5).reshape(
            N, C, 36, FREE).reshape(N, 108, FREE)
        xim[:, 108, :] = 1.0

        pos = pos_mat[0, PR * core: PR * (core + 1), :]
        pos = pos.reshape(2, 8, 2, W, 2, 3).transpose(0, 2, 4, 1, 3, 5) \
            .reshape(PR, 3)
        posTc = np.ascontiguousarray(
            np.concatenate([pos, np.ones((PR, 1), f)], 1).T)

        rm = np.ones((128, 8), f)
        if core == 0:
            rm[:, 0] = 0.0            # ft row 0
            rm[0:64, 2] = 0.0         # fb row 0, base half
        if core == NCORES - 1:
            rm[:, 1] = 0.0            # ft row 17
            rm[:, 3] = 0.0            # fb row 17
            rm[64:128, 4] = 0.0       # fb row 16, +1row half

        in_maps.append({"xim": xim, "posT": posTc.astype(np.float16),
                        "cwr": cwr.astype(f8),
                        "w1a": w1a.astype(np.float16),
                        "w2p": w2p.reshape(3, 128, 2 * WM).astype(f8),
                        "rmask": rm, "sel3": sel3, "mean4": mean4,
                        "ones16": ones16})
    return in_maps


def _assemble(results):
    full = np.empty((N, 3, H * SCALE, W * SCALE), np.float32)
    for core in range(NCORES):
        r = results[core]["out"].reshape(N, 3, 2, 2, HS, W)
        blk = r.transpose(0, 1, 4, 2, 5, 3).reshape(N, 3, HS * 2, W * 2)
        full[:, :, HS * 2 * core: HS * 2 * (core + 1), :] = blk
    return full


def kernel(**inputs):
    from concourse.bass_utils import run_bass_kernel_spmd
    if "nc" not in _CACHE:
        _CACHE["nc"] = _build_nc()
    in_maps = _host_prep(**inputs)
    res = run_bass_kernel_spmd(_CACHE["nc"], in_maps, list(range(NCORES)))
    _CACHE["last_result"] = res
    return _assemble(res.results)


# revision 44
# speedup vs baseline: 1.0162x; 1.0138x over previous
"""MetaSR (meta-upscale CNN) Trainium2 kernel, SPMD over 8 NeuronCores.

Algorithm (bilinear reformulation of the reference):
    feat = relu(conv5x5(x) + b)                      [N,64,H,W]
    hid  = relu(pos @ w1 + b1)                       [(H*s*W*s), 256]
    out[n,p,l,c] = sum_h hid[r(p,l),h] * U[n,l,h,c] + bias[n,l,c] + mean_c
      where U[n,l,h,c] = sum_k cols[n,l,k] * w2[h, k*3+c]   (k = 3x3 taps x 64 ch)
            bias[n,l,c] = sum_k cols[n,l,k] * b2[k*3+c]

Sharding: 8 horizontal strips of 16 image rows each (all of N on every core).

v5 design (176us/core vs 489us baseline):
  - host-side 6x6 im2col (+ones row for the folded conv bias): one [109,2340]
    fp16 DMA per image; conv = plain fp16 matmuls producing ft = [base;+1col]
    and fb = [base;+1row] directly (128 rows each) into one fp8 F tile.
    Halo zeroing via col memsets + per-partition row-mask multiplies.
  - MLP layer 1: posT [4,8192] fp16 loaded whole; plain fp16 matmuls.
  - stage B (U^T [128h, pix] psum tiles, fp8 DoubleRow): 3 matmuls per
    512-pix tile: k-tile pairs (taps01+34), (taps67+25), (tap8+zero) via
    strided 2nd-k-tile access patterns into the combined [ft|fb] tile.
  - per-pixel bias: compact M=4 DoubleRow matmuls -> [4,512] psum; ACT
    eviction adds the RGB means (per-partition bias) -> bs fp16; a K=4
    selector matmul broadcasts row cc of bs onto the output psum, then 8
    fp16 ones-matmuls (tile_position col packing) accumulate the DVE
    product pt = us * hidT reduced over h.
"""
import os
import numpy as np

SCALE = 2
RGB_MEAN = (0.4488, 0.4371, 0.404)
N, C, H, W = 4, 3, 128, 128
G0 = 64
NCORES = 8
HS = H // NCORES          # image rows per core (16)
FR = HS + 2               # feat rows incl unfold halo (18)
FC = W + 2                # feat cols incl unfold halo (130)
FREE = FR * FC            # 2340
XR = HS + 7               # padded x rows per core (23)
XW = W + 7                # padded x cols (135)
HH = 256                  # MLP hidden
LP = HS * W               # pixels per core (2048)
PR = 4 * LP               # pos rows per core (8192)
KC = 109                  # conv im2col K (3*36 taps + ones row)
WM = 800                  # w2p M cols: 6x128 U-blocks + 32 (4 bias + pad)

_CACHE = {}


def _build_nc():
    import concourse.bass as bass
    import concourse.tile as tile
    from concourse import bacc, mybir

    f32 = mybir.dt.float32
    f16 = mybir.dt.float16
    f8 = mybir.dt.float8e4
    DR = mybir.MatmulPerfMode.DoubleRow

    nc = bacc.Bacc("TRN2", target_bir_lowering=False, debug=False,
                   num_devices=NCORES)

    xim = nc.dram_tensor("xim", [N, KC, FREE], f8, kind="ExternalInput").ap()
    posT = nc.dram_tensor("posT", [4, PR], f16, kind="ExternalInput").ap()
    cwr = nc.dram_tensor("cwr", [2, KC, 128], f8, kind="ExternalInput").ap()
    w1a = nc.dram_tensor("w1a", [4, HH], f16, kind="ExternalInput").ap()
    w2p = nc.dram_tensor("w2p", [3, 128, 2 * WM], f8,
                         kind="ExternalInput").ap()
    rmask = nc.dram_tensor("rmask", [128, 8], f32, kind="ExternalInput").ap()
    sel3 = nc.dram_tensor("sel3", [4, 384], f16, kind="ExternalInput").ap()
    mean4 = nc.dram_tensor("mean4", [4, 1], f32, kind="ExternalInput").ap()
    ones16 = nc.dram_tensor("ones16", [128, 32], f16,
                            kind="ExternalInput").ap()
    out = nc.dram_tensor("out", [N, 3, 4, LP], f32, kind="ExternalOutput").ap()

    with tile.TileContext(nc) as tc:
        with tc.tile_pool(name="const", bufs=1) as cpool, \
             tc.tile_pool(name="feat", bufs=1) as fpool, \
             tc.tile_pool(name="hid", bufs=1) as hpool, \
             tc.tile_pool(name="im2col", bufs=2) as xpool, \
             tc.tile_pool(name="usb", bufs=5) as upool, \
             tc.tile_pool(name="pt", bufs=8) as ppool, \
             tc.tile_pool(name="bs", bufs=4) as bspool, \
             tc.tile_pool(name="ob", bufs=4) as obpool, \
             tc.tile_pool(name="bps", bufs=5, space="PSUM") as bps, \
             tc.tile_pool(name="ops", bufs=3, space="PSUM") as ops:

            # ---- PE pstate warmup: dummy matmuls with no DMA deps so the
            # array is at full clock when the real work's data arrives ----
            warm = cpool.tile([128, 512], f16, tag="warm")
            nc.vector.memset(warm[:], 1.0)
            for _ in range(10):
                wps = bps.tile([128, 512], f32, tag="B")
                nc.tensor.matmul(wps[:], warm[:, 0:128], warm[:],
                                 start=True, stop=True)

            # ---- constants to SBUF ----
            w1a_t = cpool.tile([4, HH], f16, tag="w1a")
            nc.scalar.dma_start(w1a_t[:], w1a[:])
            posT_t = cpool.tile([4, PR], f16, tag="posT")
            for ci, eng in enumerate((nc.scalar, nc.gpsimd, nc.scalar,
                                      nc.gpsimd)):
                eng.dma_start(posT_t[:, ci * 2048:(ci + 1) * 2048],
                              posT[:, ci * 2048:(ci + 1) * 2048])
            cwr_t = []
            for v in range(2):
                t = cpool.tile([KC, 128], f8, tag=f"cwr{v}")
                nc.scalar.dma_start(t[:], cwr[v])
                cwr_t.append(t)
            w2p_t = []
            for p in range(3):
                t = cpool.tile([128, 2 * WM], f8, tag=f"w2p{p}")
                nc.gpsimd.dma_start(t[:], w2p[p])
                w2p_t.append(t)
            rmask_t = cpool.tile([128, 8], f32, tag="rmask")
            nc.gpsimd.dma_start(rmask_t[:], rmask[:])
            sel3_t = cpool.tile([4, 384], f16, tag="sel3")
            nc.gpsimd.dma_start(sel3_t[:], sel3[:])
            mean4_t = cpool.tile([4, 1], f32, tag="mean4")
            nc.gpsimd.dma_start(mean4_t[:], mean4[:])
            ones_t = cpool.tile([128, 32], f16, tag="ones16")
            nc.gpsimd.dma_start(ones_t[:], ones16[:])

            # ---- MLP layer 1 -> hidT fp16 tiles [128 h, 4096 (p,pix)] ----
            # posT column order (host): lp*4096 + p*1024 + (l % 1024)
            hidT = [[None] * 2, [None] * 2]
            for lp in range(2):
                for hch in range(2):
                    hb = hpool.tile([128, 4096], f16, tag=f"hid{hch}_{lp}")
                    for cb in range(8):
                        ps = bps.tile([128, 512], f32, tag="B")
                        o = lp * 4096 + cb * 512
                        nc.tensor.matmul(
                            ps[:], w1a_t[:, hch * 128:(hch + 1) * 128],
                            posT_t[:, o:o + 512],
                            start=True, stop=True)
                        dst = hb[:, cb * 512:(cb + 1) * 512]
                        if (cb + hch) % 2 == 0:
                            nc.scalar.activation(
                                dst, ps[:],
                                mybir.ActivationFunctionType.Relu)
                        else:
                            nc.vector.tensor_scalar_max(dst, ps[:], 0.0)
                    hidT[hch][lp] = hb

            # ---- conv -> F = [ft | fb] fp8 ----
            dmae = [nc.sync, nc.vector, nc.scalar, nc.gpsimd]
            feat = []
            for n in range(N):
                F = fpool.tile([128, 2 * FREE], f8, tag=f"feat{n}")
                xt = xpool.tile([KC, FREE], f8, tag="x")
                for (p0, p1) in ((0, 32), (32, 64), (64, 96), (96, KC)):
                    nc.sync.dma_start(xt[p0:p1, :], xim[n][p0:p1, :])

                for v in range(2):  # 0: ft (base|+1col), 1: fb (base|+1row)
                    for ch in range(5):
                        o = ch * 512
                        nw = min(512, FREE - o)
                        ps = bps.tile([128, 512], f32, tag="B")
                        nc.tensor.matmul(ps[:, :nw], cwr_t[v][:],
                                         xt[:, o:o + nw],
                                         start=True, stop=True)
                        dst = F[:, v * FREE + o: v * FREE + o + nw]
                        if (ch + v) % 2 == 0:
                            nc.scalar.activation(
                                dst, ps[:, :nw],
                                mybir.ActivationFunctionType.Relu)
                        else:
                            nc.vector.tensor_scalar_max(dst, ps[:, :nw], 0.0)

                # halo zeroing: columns via memsets
                fa = F[:]
                flo = F[0:64, :]
                fhi = F[64:128, :]
                # ft lower: cols {0,129}; ft upper: cols {128,129}
                nc.gpsimd.memset(bass.AP(flo.tensor, flo.offset,
                                         [flo.ap[0], [FC, FR], [FC - 1, 2]]), 0)
                nc.gpsimd.memset(bass.AP(fhi.tensor, fhi.offset + FC - 2,
                                         [fhi.ap[0], [FC, FR], [1, 2]]), 0)
                # fb (both halves): cols {0,129}
                nc.gpsimd.memset(bass.AP(fa.tensor, fa.offset + FREE,
                                         [fa.ap[0], [FC, FR], [FC - 1, 2]]), 0)
                # halo rows via per-core per-partition masks
                fv = fa.rearrange("p (x q) -> p x q", q=FC)
                for (row, mcol) in ((0, 0), (FR - 1, 1),
                                    (FR, 2), (2 * FR - 1, 3), (2 * FR - 2, 4)):
                    nc.vector.tensor_scalar_mul(
                        fv[:, row:row + 1, :], fv[:, row:row + 1, :],
                        rmask_t[:, mcol:mcol + 1])
                feat.append(F)

            # stage-B / bias window pair APs into F = [ft | fb]
            def wpair(n, pr, r0):
                fa = feat[n][:]
                if pr == 0:
                    off, dlt = r0 * FC, FC
                elif pr == 1:
                    off, dlt = (r0 + 2) * FC, FREE + 2 - 2 * FC
                else:
                    # i=1 k-tile has zero weights; +FC keeps the read
                    # in-bounds (spills into the fb region for r0=12)
                    off, dlt = (r0 + 2) * FC + 2, FC
                return bass.AP(fa.tensor, fa.offset + off,
                               [fa.ap[0], [dlt, 2], [FC, 4], [1, 128]])

            w2v = [w2p_t[p][:].rearrange("p (i m) -> p i m", i=2)
                   for p in range(3)]

            # ---- main loop ----
            for n in range(N):
                for lp in range(2):
                    # per-pixel bias -> bs fp16 [4, 512] x2 (mean folded in)
                    bss = []
                    for half in range(2):
                        r0 = lp * 8 + half * 4
                        pb = bps.tile([128, 512], f32, tag="B")
                        for pr in range(3):
                            nc.tensor.matmul(
                                pb[0:32, :], w2v[pr][:, :, 768:800],
                                wpair(n, pr, r0),
                                start=(pr == 0), stop=(pr == 2),
                                perf_mode=DR)
                        bs = bspool.tile([4, 512], f16, tag="bs")
                        # bias+mean is always positive (mean >= 103, |bias|
                        # < ~3) so Relu is an identity; Copy rejects AP bias
                        nc.scalar.activation(
                            bs[:], pb[0:4, :],
                            mybir.ActivationFunctionType.Relu,
                            bias=mean4_t[:])
                        bss.append(bs)

                    pts = {}
                    for cc in range(3):
                        for hch in range(2):
                            mb = cc * 2 + hch
                            us = upool.tile([128, 1024], f16, tag="us")
                            for half in range(2):
                                r0 = lp * 8 + half * 4
                                pbk = bps.tile([128, 512], f32, tag="B")
                                for pr in range(3):
                                    nc.tensor.matmul(
                                        pbk[:],
                                        w2v[pr][:, :, mb * 128:(mb + 1) * 128],
                                        wpair(n, pr, r0),
                                        start=(pr == 0), stop=(pr == 2),
                                        perf_mode=DR)
                                nc.scalar.activation(
                                    us[:, half * 512:(half + 1) * 512],
                                    pbk[:],
                                    mybir.ActivationFunctionType.Copy)
                            pt = ppool.tile([128, 4096], f16, tag="pt")
                            nc.vector.tensor_mul(
                                pt[:].rearrange("p (a q) -> p a q", q=1024),
                                us[:].unsqueeze(1).broadcast_to((128, 4, 1024)),
                                hidT[hch][lp][:].rearrange(
                                    "p (a q) -> p a q", q=1024))
                            pts[(cc, hch)] = pt

                    for cc in range(3):
                        for half in range(2):
                            po = ops.tile([128, 512], f32, tag="po")
                            nc.tensor.matmul(
                                po[:], sel3_t[:, cc * 128:(cc + 1) * 128],
                                bss[half][:],
                                start=True, stop=False,
                                skip_group_check=True)
                            for hch in range(2):
                                for p in range(4):
                                    sl = slice(p * 1024 + half * 512,
                                               p * 1024 + half * 512 + 512)
                                    nc.tensor.matmul(
                                        po[32 * p:32 * p + 32, :],
                                        ones_t[:, 0:32],
                                        pts[(cc, hch)][:, sl],
                                        start=False,
                                        stop=(hch == 1 and p == 3),
                                        skip_group_check=True,
                                        tile_position=(0, 32 * p))
                            posb = obpool.tile([128, 512], f32, tag="posb")
                            if cc % 2 == 0:
                                nc.scalar.activation(
                                    posb[:], po[:],
                                    mybir.ActivationFunctionType.Copy)
                            else:
                                nc.vector.tensor_scalar_add(
                                    posb[:], po[:], 0.0)
                            posrc = posb[:].rearrange(
                                "(a b) q -> a b q", b=32)[:, 0, :]
                            oeng = nc.sync if (cc + half) % 2 == 0 \
                                else nc.gpsimd
                            oeng.dma_start(
                                out[n, cc][:, lp * 1024 + half * 512:
                                           lp * 1024 + half * 512 + 512],
                                posrc)

    nc.compile()
    return nc


def _host_prep(x, pos_mat, conv_w, conv_b, w1, b1, w2, b2):
    from concourse import mybir
    f = np.float32
    f8 = mybir.dt.np(mybir.dt.float8e4)

    # x: pad 3 top/left, 4 bottom/right for the 6x6 tap window
    xpad = np.pad(x, ((0, 0), (0, 0), (3, 4), (3, 4))).astype(f)

    # conv weights: [v, k(109), 128]; cols = [base(64) | shifted(64)];
    # row 108 = conv bias (pairs with the im2col ones row)
    def cw6(dr, dc):
        w6 = np.zeros((G0, C, 6, 6), f)
        w6[:, :, dr:dr + 5, dc:dc + 5] = conv_w
        return w6.transpose(1, 2, 3, 0).reshape(108, G0)

    base = cw6(0, 0)
    cwr = np.zeros((2, KC, 128), f)
    for v, shifted in enumerate((cw6(0, 1), cw6(1, 0))):
        cwr[v, :108, 0:G0] = base
        cwr[v, :108, G0:128] = shifted
        cwr[v, 108, 0:G0] = conv_b
        cwr[v, 108, G0:128] = conv_b

    w1a = np.vstack([w1, b1[None, :]]).astype(f)

    # w2 pair blocks [3, 128, 2(i), 772]
    Wr = w2.reshape(HH, 576, 3)
    b2r = b2.reshape(576, 3)

    def block_mat(tlo, thi):
        M = np.zeros((128, WM), f)
        for hf, t in ((0, tlo), (1, thi)):
            if t is None:
                continue
            kidx = np.arange(G0) * 9 + t
            for mb in range(6):
                cc, hch = mb // 2, mb % 2
                M[hf * 64:(hf + 1) * 64, mb * 128:(mb + 1) * 128] = \
                    Wr[hch * 128:(hch + 1) * 128, kidx, cc].T
            M[hf * 64:(hf + 1) * 64, 768:771] = b2r[kidx, :]
        return M

    BLOCKS = {0: (0, 1), 1: (3, 4), 2: (6, 7), 3: (2, 5), 4: (8, None)}
    PAIRS = [(0, 1), (2, 3), (4, None)]
    w2p = np.zeros((3, 128, 2, WM), f)
    for P, (b0, b1_) in enumerate(PAIRS):
        w2p[P, :, 0, :] = block_mat(*BLOCKS[b0])
        if b1_ is not None:
            w2p[P, :, 1, :] = block_mat(*BLOCKS[b1_])

    sel3 = np.zeros((4, 384), np.float16)
    for cc in range(3):
        sel3[cc, cc * 128:(cc + 1) * 128] = 1.0
    mean4 = np.zeros((4, 1), f)
    mean4[:3, 0] = np.asarray(RGB_MEAN, f) * 255.0
    ones16 = np.ones((128, 32), np.float16)

    in_maps = []
    for core in range(NCORES):
        xsl = xpad[:, :, HS * core: HS * core + XR, :]
        sw = np.lib.stride_tricks.sliding_window_view(
            xsl, (FR, FC), axis=(2, 3))          # [N, C, 6, 6, FR, FC]
        xim = np.empty((N, KC, FREE), f8)
        xim[:, :108, :] = sw.transpose(0, 1, 2, 3, 4, 5).reshape(
            N, C, 36, FREE).reshape(N, 108, FREE)
        xim[:, 108, :] = 1.0

        pos = pos_mat[0, PR * core: PR * (core + 1), :]
        pos = pos.reshape(2, 8, 2, W, 2, 3).transpose(0, 2, 4, 1, 3, 5) \
            .reshape(PR, 3)
        posTc = np.ascontiguousarray(
            np.concatenate([pos, np.ones((PR, 1), f)], 1).T)

        rm = np.ones((128, 8), f)
        if core == 0:
            rm[:, 0] = 0.0            # ft row 0
            rm[0:64, 2] = 0.0         # fb row 0, base half
        if core == NCORES - 1:
            rm[:, 1] = 0.0            # ft row 17
            rm[:, 3] = 0.0            # fb row 17
            rm[64:128, 4] = 0.0       # fb row 16, +1row half

        in_maps.append({"xim": xim, "posT": posTc.astype(np.float16),
                        "cwr": cwr.astype(f8),
                        "w1a": w1a.astype(np.float16),
                        "w2p": w2p.reshape(3, 128, 2 * WM).astype(f8),
                        "rmask": rm, "sel3": sel3, "mean4": mean4,
                        "ones16": ones16})
    return in_maps


def _assemble(results):
    full = np.empty((N, 3, H * SCALE, W * SCALE), np.float32)
    for core in range(NCORES):
        r = results[core]["out"].reshape(N, 3, 2, 2, HS, W)
        blk = r.transpose(0, 1, 4, 2, 5, 3).reshape(N, 3, HS * 2, W * 2)
        full[:, :, HS * 2 * core: HS * 2 * (core + 1), :] = blk
    return full


def kernel(**inputs):
    from concourse.bass_utils import run_bass_kernel_spmd
    if "nc" not in _CACHE:
        _CACHE["nc"] = _build_nc()
    in_maps = _host_prep(**inputs)
    res = run_bass_kernel_spmd(_CACHE["nc"], in_maps, list(range(NCORES)))
    _CACHE["last_result"] = res
    return _assemble(res.results)


# revision 45
# speedup vs baseline: 1.0308x; 1.0143x over previous
"""MetaSR (meta-upscale CNN) Trainium2 kernel, SPMD over 8 NeuronCores.

Algorithm (bilinear reformulation of the reference):
    feat = relu(conv5x5(x) + b)                      [N,64,H,W]
    hid  = relu(pos @ w1 + b1)                       [(H*s*W*s), 256]
    out[n,p,l,c] = sum_h hid[r(p,l),h] * U[n,l,h,c] + bias[n,l,c] + mean_c
      where U[n,l,h,c] = sum_k cols[n,l,k] * w2[h, k*3+c]   (k = 3x3 taps x 64 ch)
            bias[n,l,c] = sum_k cols[n,l,k] * b2[k*3+c]

Sharding: 8 horizontal strips of 16 image rows each (all of N on every core).

v5 design (176us/core vs 489us baseline):
  - host-side 6x6 im2col (+ones row for the folded conv bias): one [109,2340]
    fp16 DMA per image; conv = plain fp16 matmuls producing ft = [base;+1col]
    and fb = [base;+1row] directly (128 rows each) into one fp8 F tile.
    Halo zeroing via col memsets + per-partition row-mask multiplies.
  - MLP layer 1: posT [4,8192] fp16 loaded whole; plain fp16 matmuls.
  - stage B (U^T [128h, pix] psum tiles, fp8 DoubleRow): 3 matmuls per
    512-pix tile: k-tile pairs (taps01+34), (taps67+25), (tap8+zero) via
    strided 2nd-k-tile access patterns into the combined [ft|fb] tile.
  - per-pixel bias: compact M=4 DoubleRow matmuls -> [4,512] psum; ACT
    eviction adds the RGB means (per-partition bias) -> bs fp16; a K=4
    selector matmul broadcasts row cc of bs onto the output psum, then 8
    fp16 ones-matmuls (tile_position col packing) accumulate the DVE
    product pt = us * hidT reduced over h.
"""
import os
import numpy as np

SCALE = 2
RGB_MEAN = (0.4488, 0.4371, 0.404)
N, C, H, W = 4, 3, 128, 128
G0 = 64
NCORES = 8
HS = H // NCORES          # image rows per core (16)
FR = HS + 2               # feat rows incl unfold halo (18)
FC = W + 2                # feat cols incl unfold halo (130)
FREE = FR * FC            # 2340
XR = HS + 7               # padded x rows per core (23)
XW = W + 7                # padded x cols (135)
HH = 256                  # MLP hidden
LP = HS * W               # pixels per core (2048)
PR = 4 * LP               # pos rows per core (8192)
KC = 109                  # conv im2col K (3*36 taps + ones row)
WM = 800                  # w2p M cols: 6x128 U-blocks + 32 (4 bias + pad)

_CACHE = {}


def _build_nc():
    import concourse.bass as bass
    import concourse.tile as tile
    from concourse import bacc, mybir

    f32 = mybir.dt.float32
    f16 = mybir.dt.float16
    f8 = mybir.dt.float8e4
    DR = mybir.MatmulPerfMode.DoubleRow

    nc = bacc.Bacc("TRN2", target_bir_lowering=False, debug=False,
                   num_devices=NCORES)

    xim = nc.dram_tensor("xim", [N, KC, FREE], f8, kind="ExternalInput").ap()
    posT = nc.dram_tensor("posT", [4, PR], f16, kind="ExternalInput").ap()
    cwr = nc.dram_tensor("cwr", [2, KC, 128], f8, kind="ExternalInput").ap()
    w1a = nc.dram_tensor("w1a", [4, HH], f16, kind="ExternalInput").ap()
    w2p = nc.dram_tensor("w2p", [3, 128, 2 * WM], f8,
                         kind="ExternalInput").ap()
    rmask = nc.dram_tensor("rmask", [128, 8], f32, kind="ExternalInput").ap()
    sel3 = nc.dram_tensor("sel3", [4, 384], f16, kind="ExternalInput").ap()
    mean4 = nc.dram_tensor("mean4", [4, 1], f32, kind="ExternalInput").ap()
    ones16 = nc.dram_tensor("ones16", [128, 32], f16,
                            kind="ExternalInput").ap()
    out = nc.dram_tensor("out", [N, 3, 4, LP], f32, kind="ExternalOutput").ap()

    with tile.TileContext(nc) as tc:
        with tc.tile_pool(name="const", bufs=1) as cpool, \
             tc.tile_pool(name="feat", bufs=1) as fpool, \
             tc.tile_pool(name="hid", bufs=1) as hpool, \
             tc.tile_pool(name="im2col", bufs=2) as xpool, \
             tc.tile_pool(name="usb", bufs=5) as upool, \
             tc.tile_pool(name="pt", bufs=8) as ppool, \
             tc.tile_pool(name="bs", bufs=4) as bspool, \
             tc.tile_pool(name="ob", bufs=4) as obpool, \
             tc.tile_pool(name="bps", bufs=5, space="PSUM") as bps, \
             tc.tile_pool(name="ops", bufs=3, space="PSUM") as ops:

            # ---- PE pstate warmup: dummy matmuls with no DMA deps so the
            # array is at full clock when the real work's data arrives ----
            warm = cpool.tile([128, 512], f16, tag="warm")
            nc.vector.memset(warm[:], 1.0)
            for _ in range(10):
                wps = bps.tile([128, 512], f32, tag="B")
                nc.tensor.matmul(wps[:], warm[:, 0:128], warm[:],
                                 start=True, stop=True)

            # ---- constants to SBUF ----
            w1a_t = cpool.tile([4, HH], f16, tag="w1a")
            nc.scalar.dma_start(w1a_t[:], w1a[:])
            posT_t = cpool.tile([4, PR], f16, tag="posT")
            for ci, eng in enumerate((nc.scalar, nc.gpsimd, nc.scalar,
                                      nc.gpsimd)):
                eng.dma_start(posT_t[:, ci * 2048:(ci + 1) * 2048],
                              posT[:, ci * 2048:(ci + 1) * 2048])
            cwr_t = []
            for v in range(2):
                t = cpool.tile([KC, 128], f8, tag=f"cwr{v}")
                nc.scalar.dma_start(t[:], cwr[v])
                cwr_t.append(t)
            w2p_t = []
            for p in range(3):
                t = cpool.tile([128, 2 * WM], f8, tag=f"w2p{p}")
                nc.gpsimd.dma_start(t[:], w2p[p])
                w2p_t.append(t)
            rmask_t = cpool.tile([128, 8], f32, tag="rmask")
            nc.gpsimd.dma_start(rmask_t[:], rmask[:])
            sel3_t = cpool.tile([4, 384], f16, tag="sel3")
            nc.gpsimd.dma_start(sel3_t[:], sel3[:])
            mean4_t = cpool.tile([4, 1], f32, tag="mean4")
            nc.gpsimd.dma_start(mean4_t[:], mean4[:])
            ones_t = cpool.tile([128, 32], f16, tag="ones16")
            nc.gpsimd.dma_start(ones_t[:], ones16[:])

            # ---- MLP layer 1 -> hidT fp16 tiles [128 h, 4096 (p,pix)] ----
            # posT column order (host): lp*4096 + p*1024 + (l % 1024)
            hidT = [[None] * 2, [None] * 2]
            for lp in range(2):
                for hch in range(2):
                    hb = hpool.tile([128, 4096], f16, tag=f"hid{hch}_{lp}")
                    for cb in range(8):
                        ps = bps.tile([128, 512], f32, tag="B")
                        o = lp * 4096 + cb * 512
                        nc.tensor.matmul(
                            ps[:], w1a_t[:, hch * 128:(hch + 1) * 128],
                            posT_t[:, o:o + 512],
                            start=True, stop=True)
                        dst = hb[:, cb * 512:(cb + 1) * 512]
                        if (cb + hch) % 2 == 0:
                            nc.scalar.activation(
                                dst, ps[:],
                                mybir.ActivationFunctionType.Relu)
                        else:
                            nc.vector.tensor_scalar_max(dst, ps[:], 0.0)
                    hidT[hch][lp] = hb

            # ---- conv -> F = [ft | fb] fp8 ----
            dmae = [nc.sync, nc.vector, nc.scalar, nc.gpsimd]
            feat = []
            for n in range(N):
                F = fpool.tile([128, 2 * FREE], f8, tag=f"feat{n}")
                xt = xpool.tile([KC, FREE], f8, tag="x")
                for (p0, p1) in ((0, 32), (32, 64), (64, 96), (96, KC)):
                    nc.sync.dma_start(xt[p0:p1, :], xim[n][p0:p1, :])

                for v in range(2):  # 0: ft (base|+1col), 1: fb (base|+1row)
                    for ch in range(5):
                        o = ch * 512
                        nw = min(512, FREE - o)
                        ps = bps.tile([128, 512], f32, tag="B")
                        nc.tensor.matmul(ps[:, :nw], cwr_t[v][:],
                                         xt[:, o:o + nw],
                                         start=True, stop=True)
                        dst = F[:, v * FREE + o: v * FREE + o + nw]
                        if (ch + v) % 2 == 0:
                            nc.scalar.activation(
                                dst, ps[:, :nw],
                                mybir.ActivationFunctionType.Relu)
                        else:
                            nc.vector.tensor_scalar_max(dst, ps[:, :nw], 0.0)

                # halo zeroing: columns via memsets
                fa = F[:]
                flo = F[0:64, :]
                fhi = F[64:128, :]
                # ft lower: cols {0,129}; ft upper: cols {128,129}
                nc.gpsimd.memset(bass.AP(flo.tensor, flo.offset,
                                         [flo.ap[0], [FC, FR], [FC - 1, 2]]), 0)
                nc.gpsimd.memset(bass.AP(fhi.tensor, fhi.offset + FC - 2,
                                         [fhi.ap[0], [FC, FR], [1, 2]]), 0)
                # fb (both halves): cols {0,129}
                nc.gpsimd.memset(bass.AP(fa.tensor, fa.offset + FREE,
                                         [fa.ap[0], [FC, FR], [FC - 1, 2]]), 0)
                # halo rows via per-core per-partition masks
                fv = fa.rearrange("p (x q) -> p x q", q=FC)
                for (row, mcol) in ((0, 0), (FR - 1, 1),
                                    (FR, 2), (2 * FR - 1, 3), (2 * FR - 2, 4)):
                    nc.vector.tensor_scalar_mul(
                        fv[:, row:row + 1, :], fv[:, row:row + 1, :],
                        rmask_t[:, mcol:mcol + 1])
                feat.append(F)

            # stage-B / bias window pair APs into F = [ft | fb]
            def wpair(n, pr, r0):
                fa = feat[n][:]
                if pr == 0:
                    off, dlt = r0 * FC, FC
                elif pr == 1:
                    off, dlt = (r0 + 2) * FC, FREE + 2 - 2 * FC
                else:
                    # i=1 k-tile has zero weights; +FC keeps the read
                    # in-bounds (spills into the fb region for r0=12)
                    off, dlt = (r0 + 2) * FC + 2, FC
                return bass.AP(fa.tensor, fa.offset + off,
                               [fa.ap[0], [dlt, 2], [FC, 4], [1, 128]])

            w2v = [w2p_t[p][:].rearrange("p (i m) -> p i m", i=2)
                   for p in range(3)]

            # ---- main loop ----
            for n in range(N):
                for lp in range(2):
                    # per-pixel bias -> bs fp16 [4, 512] x2 (mean folded in)
                    bss = []
                    for half in range(2):
                        r0 = lp * 8 + half * 4
                        pb = bps.tile([128, 512], f32, tag="B")
                        for pr in range(3):
                            nc.tensor.matmul(
                                pb[0:32, :], w2v[pr][:, :, 768:800],
                                wpair(n, pr, r0),
                                start=(pr == 0), stop=(pr == 2),
                                perf_mode=DR)
                        bs = bspool.tile([4, 512], f16, tag="bs")
                        # bias+mean is always positive (mean >= 103, |bias|
                        # < ~3) so Relu is an identity; Copy rejects AP bias
                        nc.scalar.activation(
                            bs[:], pb[0:4, :],
                            mybir.ActivationFunctionType.Relu,
                            bias=mean4_t[:])
                        bss.append(bs)

                    pts = {}
                    for cc in range(3):
                        for hch in range(2):
                            mb = cc * 2 + hch
                            us = upool.tile([128, 1024], f16, tag="us")
                            for half in range(2):
                                r0 = lp * 8 + half * 4
                                pbk = bps.tile([128, 512], f32, tag="B")
                                for pr in range(3):
                                    nc.tensor.matmul(
                                        pbk[:],
                                        w2v[pr][:, :, mb * 128:(mb + 1) * 128],
                                        wpair(n, pr, r0),
                                        start=(pr == 0), stop=(pr == 2),
                                        perf_mode=DR)
                                nc.scalar.activation(
                                    us[:, half * 512:(half + 1) * 512],
                                    pbk[:],
                                    mybir.ActivationFunctionType.Copy)
                            pt = ppool.tile([128, 4096], f16, tag="pt")
                            nc.vector.tensor_mul(
                                pt[:].rearrange("p (a q) -> p a q", q=1024),
                                us[:].unsqueeze(1).broadcast_to((128, 4, 1024)),
                                hidT[hch][lp][:].rearrange(
                                    "p (a q) -> p a q", q=1024))
                            pts[(cc, hch)] = pt

                    for cc in range(3):
                        for half in range(2):
                            po = ops.tile([128, 512], f32, tag="po")
                            nc.tensor.matmul(
                                po[:], sel3_t[:, cc * 128:(cc + 1) * 128],
                                bss[half][:],
                                start=True, stop=False,
                                skip_group_check=True)
                            for hch in range(2):
                                for p in range(4):
                                    sl = slice(p * 1024 + half * 512,
                                               p * 1024 + half * 512 + 512)
                                    nc.tensor.matmul(
                                        po[32 * p:32 * p + 32, :],
                                        ones_t[:, 0:32],
                                        pts[(cc, hch)][:, sl],
                                        start=False,
                                        stop=(hch == 1 and p == 3),
                                        skip_group_check=True,
                                        tile_position=(0, 32 * p))
                            posb = obpool.tile([128, 512], f32, tag="posb")
                            if cc % 2 == 0:
                                nc.scalar.activation(
                                    posb[:], po[:],
                                    mybir.ActivationFunctionType.Copy)
                            else:
                                nc.vector.tensor_scalar_add(
                                    posb[:], po[:], 0.0)
                            posrc = posb[:].rearrange(
                                "(a b) q -> a b q", b=32)[:, 0, :]
                            oeng = nc.sync if (cc + half) % 2 == 0 \
                                else nc.scalar
                            oeng.dma_start(
                                out[n, cc][:, lp * 1024 + half * 512:
                                           lp * 1024 + half * 512 + 512],
                                posrc)

    nc.compile()
    return nc


def _host_prep(x, pos_mat, conv_w, conv_b, w1, b1, w2, b2):
    from concourse import mybir
    f = np.float32
    f8 = mybir.dt.np(mybir.dt.float8e4)

    # x: pad 3 top/left, 4 bottom/right for the 6x6 tap window
    xpad = np.pad(x, ((0, 0), (0, 0), (3, 4), (3, 4))).astype(f)

    # conv weights: [v, k(109), 128]; cols = [base(64) | shifted(64)];
    # row 108 = conv bias (pairs with the im2col ones row)
    def cw6(dr, dc):
        w6 = np.zeros((G0, C, 6, 6), f)
        w6[:, :, dr:dr + 5, dc:dc + 5] = conv_w
        return w6.transpose(1, 2, 3, 0).reshape(108, G0)

    base = cw6(0, 0)
    cwr = np.zeros((2, KC, 128), f)
    for v, shifted in enumerate((cw6(0, 1), cw6(1, 0))):
        cwr[v, :108, 0:G0] = base
        cwr[v, :108, G0:128] = shifted
        cwr[v, 108, 0:G0] = conv_b
        cwr[v, 108, G0:128] = conv_b

    w1a = np.vstack([w1, b1[None, :]]).astype(f)

    # w2 pair blocks [3, 128, 2(i), 772]
    Wr = w2.reshape(HH, 576, 3)
    b2r = b2.reshape(576, 3)

    def block_mat(tlo, thi):
        M = np.zeros((128, WM), f)
        for hf, t in ((0, tlo), (1, thi)):
            if t is None:
                continue
            kidx = np.arange(G0) * 9 + t
            for mb in range(6):
                cc, hch = mb // 2, mb % 2
                M[hf * 64:(hf + 1) * 64, mb * 128:(mb + 1) * 128] = \
                    Wr[hch * 128:(hch + 1) * 128, kidx, cc].T
            M[hf * 64:(hf + 1) * 64, 768:771] = b2r[kidx, :]
        return M

    BLOCKS = {0: (0, 1), 1: (3, 4), 2: (6, 7), 3: (2, 5), 4: (8, None)}
    PAIRS = [(0, 1), (2, 3), (4, None)]
    w2p = np.zeros((3, 128, 2, WM), f)
    for P, (b0, b1_) in enumerate(PAIRS):
        w2p[P, :, 0, :] = block_mat(*BLOCKS[b0])
        if b1_ is not None:
            w2p[P, :, 1, :] = block_mat(*BLOCKS[b1_])

    sel3 = np.zeros((4, 384), np.float16)
    for cc in range(3):
        sel3[cc, cc * 128:(cc + 1) * 128] = 1.0
    mean4 = np.zeros((4, 1), f)
    mean4[:3, 0] = np.asarray(RGB_MEAN, f) * 255.0
    ones16 = np.ones((128, 32), np.float16)

    in_maps = []
    for core in range(NCORES):
        xsl = xpad[:, :, HS * core: HS * core + XR, :]
        sw = np.lib.stride_tricks.sliding_window_view(
            xsl, (FR, FC), axis=(2, 3))          # [N, C, 6, 6, FR, FC]
        xim = np.empty((N, KC, FREE), f8)
        xim[:, :108, :] = sw.transpose(0, 1, 2, 3, 4, 5).reshape(
            N, C, 36, FREE).reshape(N, 108, FREE)
        xim[:, 108, :] = 1.0

        pos = pos_mat[0, PR * core: PR * (core + 1), :]
        pos = pos.reshape(2, 8, 2, W, 2, 3).transpose(0, 2, 4, 1, 3, 5) \
            .reshape(PR, 3)
        posTc = np.ascontiguousarray(
            np.concatenate([pos, np.ones((PR, 1), f)], 1).T)

        rm = np.ones((128, 8), f)
        if core == 0:
            rm[:, 0] = 0.0            # ft row 0
            rm[0:64, 2] = 0.0         # fb row 0, base half
        if core == NCORES - 1:
            rm[:, 1] = 0.0            # ft row 17
            rm[:, 3] = 0.0            # fb row 17
            rm[64:128, 4] = 0.0       # fb row 16, +1row half

        in_maps.append({"xim": xim, "posT": posTc.astype(np.float16),
                        "cwr": cwr.astype(f8),
                        "w1a": w1a.astype(np.float16),
                        "w2p": w2p.reshape(3, 128, 2 * WM).astype(f8),
                        "rmask": rm, "sel3": sel3, "mean4": mean4,
                        "ones16": ones16})
    return in_maps


def _assemble(results):
    full = np.empty((N, 3, H * SCALE, W * SCALE), np.float32)
    for core in range(NCORES):
        r = results[core]["out"].reshape(N, 3, 2, 2, HS, W)
        blk = r.transpose(0, 1, 4, 2, 5, 3).reshape(N, 3, HS * 2, W * 2)
        full[:, :, HS * 2 * core: HS * 2 * (core + 1), :] = blk
    return full


def kernel(**inputs):
    from concourse.bass_utils import run_bass_kernel_spmd
    if "nc" not in _CACHE:
        _CACHE["nc"] = _build_nc()
    in_maps = _host_prep(**inputs)
    res = run_bass_kernel_spmd(_CACHE["nc"], in_maps, list(range(NCORES)))
    _CACHE["last_result"] = res
    return _assemble(res.results)
